# revision 10
# baseline (speedup 1.0000x reference)
"""Trainium2 Bass kernel for nn_DGEmbedding (DimeNet-style dist/angle embedding).

8 NeuronCores SPMD: edges (1M) and triplets (2.5M) sharded 1/8 per core,
dists replicated for the irregular triplet->edge gather (per-column indirect
DMA, 128 rows/instr). Spherical-Bessel rbf recomputed per triplet via the
upward recurrence shared across the 17 distinct frequency classes
m = 2(k+1)+l in {2..18} (x_m = u*(m/2)*pi); j_l for column (l,k) is the
m-chain value at order l, m = l+2k+2 (stride-2 slice of the m axis).
cbf = Legendre P_0..P_6(cos angle); out = env(u)*j_l*cbf.
dist_embs = env(d)*sin(freq_k*d) on the edge shard (freq from input).

Numerics mirror the reference fp32 op sequence (divide by 5, f64-derived z
constants, upward recurrence) so deviation vs the fp32 reference is dominated
by the reference's own fp32 instability envelope.
"""
import numpy as np

import concourse.bass as bass
import concourse.mybir as mybir
from concourse.tile import TileContext
from concourse.bass_utils import run_bass_kernel_spmd

F32 = mybir.dt.float32
I32 = mybir.dt.int32
AL = mybir.AluOpType
AF = mybir.ActivationFunctionType

N_CORES = 8
E_FULL = 1_000_000
T_FULL = 2_500_000
E_SH = E_FULL // N_CORES          # 125000
T_SH = T_FULL // N_CORES          # 312500
FE = 1024                          # padded per-partition edge run: 128*1024
FT = 2560                          # padded per-partition triplet run: 128*2560
E_PAD = 128 * FE
T_PAD = 128 * FT

_MI = np.arange(2, 19)
_ZV = ((_MI / 2.0) * np.pi).astype(np.float32)      # z_m = (m/2)*pi  (f64 -> f32)
_WV = (2.0 / (_MI * np.pi)).astype(np.float32)      # 1/z_m
HALF_PI = float(np.pi / 2.0)
INV2PI = float(1.0 / (2.0 * np.pi))
# Cody-Waite 2-term split of 2*pi: C1 exact in fp32 with short mantissa
_C1 = float(np.float32(6.28125))
_C2 = float(2.0 * np.pi - 6.28125)


def _rsin(nc, qt, qit, out_ap, x_ap, shift):
    """out = sin(x + shift) with range reduction to [-pi, pi].
    qt: f32 scratch AP, qit: i32 scratch AP (same shape as x)."""
    AL_ = mybir.AluOpType
    nc.vector.tensor_scalar(qt, x_ap, INV2PI, shift * INV2PI, AL_.mult, AL_.add)
    nc.vector.tensor_copy(out=qit, in_=qt)        # f32 -> i32, round-nearest
    nc.vector.tensor_copy(out=qt, in_=qit)        # i32 -> f32
    # r = (x - k*C1 + shift) - k*C2
    nc.vector.tensor_scalar(out_ap, qt, -_C1, shift, AL_.mult, AL_.add)
    nc.vector.tensor_tensor(out=out_ap, in0=x_ap, in1=out_ap, op=AL_.add)
    nc.vector.tensor_scalar(qt, qt, -_C2, None, AL_.mult)
    nc.vector.tensor_tensor(out=out_ap, in0=out_ap, in1=qt, op=AL_.add)
    nc.scalar.activation(out_ap, out_ap, AF.Sin)


def _bc(ap, axis, count):
    """Insert a stride-0 broadcast dim into an AP at free-dim position `axis`
    (0 = right after partition dim)."""
    dims = list(ap.ap)
    dims.insert(1 + axis, [0, count])
    return bass.AP(ap.tensor, ap.offset, dims)


def _envelope(nc, wp, ut, vt, F, pref):
    """env = 1/x + a*p0 + b*p0*x + c*p0*x*x, p0 = x^5 (a=-28, b=48, c=21)."""
    u2 = wp.tile([128, F], F32, tag=pref + "u2")
    nc.vector.tensor_tensor(out=u2[:], in0=ut[:], in1=ut[:], op=AL.mult)
    nc.vector.tensor_tensor(out=u2[:], in0=u2[:], in1=u2[:], op=AL.mult)   # x^4
    p0 = wp.tile([128, F], F32, tag=pref + "p0")
    nc.vector.tensor_tensor(out=p0[:], in0=u2[:], in1=ut[:], op=AL.mult)   # x^5
    env = wp.tile([128, F], F32, tag=pref + "env")
    nc.vector.tensor_scalar(env[:], p0[:], -28.0, None, AL.mult)
    nc.vector.tensor_tensor(out=env[:], in0=vt[:], in1=env[:], op=AL.add)
    nc.vector.tensor_tensor(out=p0[:], in0=p0[:], in1=ut[:], op=AL.mult)   # x^6
    t2 = wp.tile([128, F], F32, tag=pref + "t2")
    nc.vector.tensor_scalar(t2[:], p0[:], 48.0, None, AL.mult)
    nc.vector.tensor_tensor(out=env[:], in0=env[:], in1=t2[:], op=AL.add)
    nc.vector.tensor_tensor(out=p0[:], in0=p0[:], in1=ut[:], op=AL.mult)   # x^7
    nc.vector.tensor_scalar(t2[:], p0[:], 21.0, None, AL.mult)
    nc.vector.tensor_tensor(out=env[:], in0=env[:], in1=t2[:], op=AL.add)
    return env


def build(ft_blk=128, n_blocks=None, fe_blk=512, n_eblocks=None):
    if n_blocks is None:
        n_blocks = FT // ft_blk
    if n_eblocks is None:
        n_eblocks = FE // fe_blk
    nc = bass.Bass(trn_type="TRN2")

    d_full = nc.dram_tensor("d_full", [E_FULL, 1], F32, kind="ExternalInput")
    d_sh = nc.dram_tensor("d_sh", [E_PAD], F32, kind="ExternalInput")
    ang = nc.dram_tensor("ang", [T_PAD], F32, kind="ExternalInput")
    kj = nc.dram_tensor("kj", [T_PAD], I32, kind="ExternalInput")
    frq = nc.dram_tensor("frq", [128, 6], F32, kind="ExternalInput")
    zr = nc.dram_tensor("zr", [128, 17], F32, kind="ExternalInput")
    wr = nc.dram_tensor("wr", [128, 17], F32, kind="ExternalInput")
    de = nc.dram_tensor("de", [E_PAD, 6], F32, kind="ExternalOutput")
    out = nc.dram_tensor("out", [T_PAD, 42], F32, kind="ExternalOutput")

    d_sh2 = d_sh[:].rearrange("(p f) -> p f", p=128)
    ang2 = ang[:].rearrange("(p f) -> p f", p=128)
    kj2 = kj[:].rearrange("(p f) -> p f", p=128)
    de2 = de[:].rearrange("(p f) k -> p f k", p=128)
    out2 = out[:].rearrange("(p f) c -> p f c", p=128)

    with TileContext(nc) as tc:
        with (
            tc.tile_pool(name="const", bufs=1) as cp,
            tc.tile_pool(name="work", bufs=2) as wp,
            tc.tile_pool(name="wide", bufs=1) as bp,
        ):
            frq_t = cp.tile([128, 6], F32)
            nc.sync.dma_start(out=frq_t[:], in_=frq[:])
            zr_t = cp.tile([128, 17], F32)
            nc.sync.dma_start(out=zr_t[:], in_=zr[:])
            wr_t = cp.tile([128, 17], F32)
            nc.sync.dma_start(out=wr_t[:], in_=wr[:])
            hp_t = cp.tile([128, 1], F32)
            nc.vector.memset(hp_t[:], HALF_PI)

            # ---------------- edge pass: dist_embs ----------------
            for eb in range(n_eblocks):
                F = fe_blk
                dt_ = wp.tile([128, F], F32, tag="eD")
                nc.sync.dma_start(out=dt_[:], in_=d_sh2[:, eb * F:(eb + 1) * F])
                ut = wp.tile([128, F], F32, tag="eU")
                nc.vector.tensor_scalar(ut[:], dt_[:], 0.2, None, AL.mult)
                vt = wp.tile([128, F], F32, tag="eV")
                nc.vector.reciprocal(vt[:], ut[:])
                env = _envelope(nc, wp, ut, vt, F, "e")
                # layout (f, k): X6[p, f, k] = u[p,f] * freq[k]
                x6 = bp.tile([128, F * 6], F32, tag="eX6")
                x6v = x6[:].rearrange("p (f k) -> p f k", k=6)
                nc.vector.tensor_tensor(out=x6v, in0=_bc(ut[:], 1, 6),
                                        in1=_bc(frq_t[:], 0, F), op=AL.mult)
                q6 = bp.tile([128, F * 6], F32, tag="eQ6")
                qi6 = bp.tile([128, F * 6], I32, tag="eQI6")
                s6 = bp.tile([128, F * 6], F32, tag="eS6")
                s6v = s6[:].rearrange("p (f k) -> p f k", k=6)
                _rsin(nc, q6[:], qi6[:], s6[:], x6[:], 0.0)
                nc.vector.tensor_tensor(out=s6v, in0=s6v, in1=_bc(env[:], 1, 6),
                                        op=AL.mult)
                nc.sync.dma_start(out=de2[:, eb * F:(eb + 1) * F, :], in_=s6v)

            # ---------------- triplet pass ----------------
            for blk in range(n_blocks):
                F = ft_blk
                kjt = wp.tile([128, F], I32, tag="tKJ")
                nc.sync.dma_start(out=kjt[:], in_=kj2[:, blk * F:(blk + 1) * F])
                angt = wp.tile([128, F], F32, tag="tANG")
                nc.sync.dma_start(out=angt[:], in_=ang2[:, blk * F:(blk + 1) * F])

                g = wp.tile([128, F], F32, tag="tG")
                for i in range(F):
                    nc.gpsimd.indirect_dma_start(
                        out=g[:, i:i + 1], out_offset=None, in_=d_full[:],
                        in_offset=bass.IndirectOffsetOnAxis(ap=kjt[:, i:i + 1], axis=0),
                    )
                ut = wp.tile([128, F], F32, tag="tU")
                nc.vector.tensor_scalar(ut[:], g[:], 0.2, None, AL.mult)
                vt = wp.tile([128, F], F32, tag="tV")
                nc.vector.reciprocal(vt[:], ut[:])
                env = _envelope(nc, wp, ut, vt, F, "t")

                # m-outer wide tiles [128, 17*F]
                x = bp.tile([128, 17 * F], F32, tag="tX")
                xv = x[:].rearrange("p (m f) -> p m f", m=17)
                nc.vector.tensor_tensor(out=xv, in0=_bc(ut[:], 0, 17),
                                        in1=_bc(zr_t[:], 1, F), op=AL.mult)
                qw = bp.tile([128, 17 * F], F32, tag="tQW")
                qiw = bp.tile([128, 17 * F], I32, tag="tQIW")
                sin = bp.tile([128, 17 * F], F32, tag="tSIN")
                _rsin(nc, qw[:], qiw[:], sin[:], x[:], 0.0)
                cos = bp.tile([128, 17 * F], F32, tag="tCOS")
                _rsin(nc, qw[:], qiw[:], cos[:], x[:], HALF_PI)
                w = bp.tile([128, 17 * F], F32, tag="tW")
                nc.vector.tensor_tensor(out=w[:].rearrange("p (m f) -> p m f", m=17),
                                        in0=_bc(vt[:], 0, 17),
                                        in1=_bc(wr_t[:], 1, F), op=AL.mult)

                j0 = bp.tile([128, 17 * F], F32, tag="tJ0")
                nc.vector.tensor_tensor(out=j0[:], in0=sin[:], in1=w[:], op=AL.mult)
                j1 = bp.tile([128, 17 * F], F32, tag="tJ1")
                nc.vector.tensor_tensor(out=j1[:], in0=j0[:], in1=cos[:], op=AL.subtract)
                nc.vector.tensor_tensor(out=j1[:], in0=j1[:], in1=w[:], op=AL.mult)

                tab = bp.tile([128, 42 * F], F32, tag="tTAB")
                tabv = tab[:].rearrange("p (f c) -> p f c", c=42)

                def extract(jt, l):
                    # TAB[:, f, 6l:6l+6] = chain value at m-cols l, l+2, .., l+10
                    base = jt[:]
                    src = bass.AP(base.tensor, base.offset + l * F,
                                  [base.ap[0], [1, F], [2 * F, 6]])
                    nc.vector.tensor_copy(out=tabv[:, :, 6 * l:6 * l + 6], in_=src)

                extract(j0, 0)
                extract(j1, 1)
                j2 = bp.tile([128, 17 * F], F32, tag="tJ2")
                tiles = [j0, j1, j2]    # rotate: (prev, cur, next)
                ip, ic, inx = 0, 1, 2
                for st in range(1, 6):
                    l_new = st + 1
                    off = l_new * F
                    n = (17 - l_new) * F
                    jp_s = tiles[ip][:, off:off + n]
                    jc_s = tiles[ic][:, off:off + n]
                    jn_s = tiles[inx][:, off:off + n]
                    w_s = w[:, off:off + n]
                    nc.vector.tensor_tensor(out=jn_s, in0=jc_s, in1=w_s, op=AL.mult)
                    nc.vector.tensor_scalar(jn_s, jn_s, float(2 * st + 1), None, AL.mult)
                    nc.vector.tensor_tensor(out=jn_s, in0=jn_s, in1=jp_s, op=AL.subtract)
                    extract(tiles[inx], l_new)
                    ip, ic, inx = ic, inx, ip

                # ---- Legendre cbf ----
                c1 = wp.tile([128, F], F32, tag="tC1")
                qn = wp.tile([128, F], F32, tag="tQN")
                qin = wp.tile([128, F], I32, tag="tQIN")
                _rsin(nc, qn[:], qin[:], c1[:], angt[:], HALF_PI)
                cbf = bp.tile([128, 7 * F], F32, tag="tCBF")
                nc.vector.memset(cbf[:, 0:F], 1.0)
                nc.vector.tensor_copy(out=cbf[:, F:2 * F], in_=c1[:])
                tmp = wp.tile([128, F], F32, tag="tTMP")
                for l in range(1, 6):
                    pl = cbf[:, l * F:(l + 1) * F]
                    plm = cbf[:, (l - 1) * F:l * F]
                    pn = cbf[:, (l + 1) * F:(l + 2) * F]
                    nc.vector.tensor_tensor(out=tmp[:], in0=c1[:], in1=pl, op=AL.mult)
                    nc.vector.tensor_scalar(tmp[:], tmp[:], float(2 * l + 1), None, AL.mult)
                    nc.vector.tensor_scalar(pn, plm, float(l), None, AL.mult)
                    nc.vector.tensor_tensor(out=pn, in0=tmp[:], in1=pn, op=AL.subtract)
                    nc.vector.tensor_scalar(pn, pn, float(1.0 / (l + 1)), None, AL.mult)
                # EC = env * cbf
                nc.vector.tensor_tensor(out=cbf[:].rearrange("p (l f) -> p l f", l=7),
                                        in0=cbf[:].rearrange("p (l f) -> p l f", l=7),
                                        in1=_bc(env[:], 0, 7), op=AL.mult)
                # out[l-group] = TAB * EC_l
                for l in range(7):
                    ec = cbf[:, l * F:(l + 1) * F]
                    nc.vector.tensor_tensor(out=tabv[:, :, 6 * l:6 * l + 6],
                                            in0=tabv[:, :, 6 * l:6 * l + 6],
                                            in1=_bc(ec, 1, 6), op=AL.mult)
                nc.sync.dma_start(out=out2[:, blk * F:(blk + 1) * F, :], in_=tabv)

    _split_excess_waits(nc)
    return nc


def _split_excess_waits(nc, max_waits=1):
    """walrus codegen in this container allows only 1 sem-wait per
    instruction; move excess waits onto preceding InstNoOps."""
    ctr = 0
    for f in nc.m.functions:
        for blk in f.blocks:
            insts = blk.instructions
            out = []
            changed = False
            for inst in insts:
                si = inst.sync_info
                if si is not None and si.on_wait and len(si.on_wait) > max_waits:
                    waits = list(si.on_wait)
                    while len(waits) > max_waits:
                        chunk, waits = waits[:max_waits], waits[max_waits:]
                        ctr += 1
                        nop = mybir.InstNoOp(
                            name=f"I-waitsplit-{ctr}",
                            engine=inst.engine,
                            ins=[], outs=[],
                            sync_info=mybir.SyncInfo(on_wait=chunk, on_update=[]),
                        )
                        out.append(nop)
                        changed = True
                    si.on_wait = waits
                    inst.sync_info = si
                out.append(inst)
            if changed:
                blk.instructions = out
    return ctr


_NC_CACHE = {}


def _get_nc(key=("full",)):
    if key not in _NC_CACHE:
        if key == ("full",):
            _NC_CACHE[key] = build()
        else:
            _NC_CACHE[key] = build(*key)
    return _NC_CACHE[key]


def _prep_in_maps(dists, angles, kj, freq):
    dists = np.ascontiguousarray(np.asarray(dists, dtype=np.float32))
    angles = np.ascontiguousarray(np.asarray(angles, dtype=np.float32))
    kj = np.ascontiguousarray(np.asarray(kj)).astype(np.int32)
    freq = np.ascontiguousarray(np.asarray(freq, dtype=np.float32))
    d_full = dists.reshape(E_FULL, 1)
    frq = np.tile(freq[None, :], (128, 1))
    zrr = np.tile(_ZV[None, :], (128, 1))
    wrr = np.tile(_WV[None, :], (128, 1))
    in_maps = []
    for c in range(N_CORES):
        dsh = np.full(E_PAD, 1.0, np.float32)
        dsh[:E_SH] = dists[c * E_SH:(c + 1) * E_SH]
        ash = np.zeros(T_PAD, np.float32)
        ash[:T_SH] = angles[c * T_SH:(c + 1) * T_SH]
        ksh = np.zeros(T_PAD, np.int32)
        ksh[:T_SH] = kj[c * T_SH:(c + 1) * T_SH]
        in_maps.append({
            "d_full": d_full, "d_sh": dsh, "ang": ash, "kj": ksh,
            "frq": frq, "zr": zrr, "wr": wrr,
        })
    return in_maps


def kernel(dists, angles, kj, freq):
    nc = _get_nc()
    in_maps = _prep_in_maps(dists, angles, kj, freq)
    res = run_bass_kernel_spmd(nc, in_maps, core_ids=list(range(N_CORES)))
    de = np.concatenate([r["de"][:E_SH] for r in res.results], axis=0)
    out = np.concatenate([r["out"][:T_SH] for r in res.results], axis=0)
    return de, out
